# revision 2
# baseline (speedup 1.0000x reference)
"""ConvCapsule Trainium2 kernel.

Full inputs -> 8-way batch-parallel (over output batch b) -> full output.

Math (per core, b = core id):
  img j in 0..7:  votes[j] = conv3x3_SAME(x[j, :, :, b, :], W)  -> [32,32,256]
  preact1 = (1/16) * sum_j votes[j] + bias          (softmax of zero logits = 1/16)
  act1    = squash(preact1)   [squash over dc groups of 16]
  logits[j, s, nc] = sum_dc votes[j][s, nc, dc] * act1[s, nc, dc]
  route   = softmax(logits over nc)
  preact2 = sum_j route[j] * votes[j] + bias
  out     = squash(preact2)

The end-to-end call is dominated by the axon-tunnel transfer (~35 MB/s), so
the host<->device traffic is minimized:
  - one bf16 input buffer per core [201, 1024]: rows 0..127 = x slice
    (img*ch channel-major), rows 128..200 = packed conv weight tables
    (identical on every core), so the im2col expansion happens ON DEVICE
    via strided SBUF->SBUF DMAs instead of shipping a 4 MB/core S tensor.
  - output is written as bf16 and upcast on host.
  - the jax.jit(shard_map) executable is built once and cached; donated
    output buffers are created on-device (no 8 MB zero upload per call).

Device mapping (per core):
  - conv as 2 accumulated matmuls (K=96/97 + K=48) per 128-pixel chunk per
    image, reading 6 w/h-shifted channel groups from the on-device-built S
    tile (zero padded, ones row for fused bias).
  - preact1 via duplicate matmuls with W/16-scaled weights accumulating in
    PSUM.
  - routing on DVE/ACT/GPSIMD; squash factor applied after the grouped
    reduce (linearity).
"""

import numpy as np

import jax
import jax.numpy as jnp
from jax.sharding import Mesh, NamedSharding, PartitionSpec
from jax.experimental.shard_map import shard_map

import concourse.bacc as bacc
import concourse.tile as tile
from concourse import mybir
from concourse import bass2jax

F32 = mybir.dt.float32
BF16 = mybir.dt.bfloat16
NP_BF16 = mybir.dt.np(mybir.dt.bfloat16)
AF = mybir.ActivationFunctionType
OP = mybir.AluOpType

B, H, W_, NIN, DIN = 8, 32, 32, 8, 16
NC, DC = 16, 16
O = NC * DC           # 256 out channels
SF = 36 * 32          # S free dim: 34 zero-padded rows of 32, + 2 rows tail
EPS = 1e-9
NCHUNK = 8            # spatial chunks of 128 pixels (4 rows)
NCORES = 8
GPSIMD_DMULTS = 4     # how many of the 8 route*votes products go to GPSIMD
GPSIMD_BMULT = True   # B-product on gpsimd

# packed weight rows in the 256-wide view: wc96 | wc48 | wc96s | wc48s | b
WR96, WR48, WR96S, WR48S, WRB = 0, 96, 144, 241, 289
WROWS = 290                      # total packed rows
WSLAB = (WROWS * O + 1023) // 1024   # 73 rows of 1024
XROWS = 128 + WSLAB              # 201 input rows per core
SHIFTS = [(-1, -1), (-1, 0), (-1, 1), (0, -1), (0, 0), (0, 1)]

_CACHE = {}


def build_module():
    nc = bacc.Bacc("TRN2", target_bir_lowering=False, debug=False)

    xin = nc.dram_tensor("xin", [XROWS, 1024], BF16, kind="ExternalInput")
    out = nc.dram_tensor("out", [H * W_, O], BF16, kind="ExternalOutput")
    # 256-wide view of the packed weight slab at row 128 (flat offset 128*1024)
    wv = xin.ap().rearrange("p (q r) -> (p q) r", r=O)
    WOFF = 512  # 128 * 1024 / 256

    with tile.TileContext(nc) as tc:
        with (
            tc.tile_pool(name="const", bufs=1) as constp,
            tc.tile_pool(name="simg", bufs=1) as sp,
            tc.tile_pool(name="psum", bufs=1, space="PSUM") as pp,
            tc.tile_pool(name="work", bufs=2) as wp,
            tc.tile_pool(name="small", bufs=2) as smp,
        ):
            # ---- persistent loads ----
            w96 = constp.tile([96, O], BF16)
            w48 = constp.tile([48, O], BF16)
            w96s = constp.tile([97, O], BF16)
            w48s = constp.tile([48, O], BF16)
            brow = constp.tile([1, O], BF16)
            nc.sync.dma_start(w96[:], wv[WOFF + WR96:WOFF + WR96 + 96])
            nc.sync.dma_start(w48[:], wv[WOFF + WR48:WOFF + WR48 + 48])
            nc.sync.dma_start(w96s[:], wv[WOFF + WR96S:WOFF + WR96S + 97])
            nc.sync.dma_start(w48s[:], wv[WOFF + WR48S:WOFF + WR48S + 48])
            nc.sync.dma_start(brow[:], wv[WOFF + WRB:WOFF + WRB + 1])

            xall = sp.tile([128, 1024], BF16, name="xall")
            nc.sync.dma_start(xall[:], xin.ap()[0:128])

            # bias tile [128, O] = ones[128]^T (x) brow, via PE broadcast
            ones1 = constp.tile([1, 128], BF16)
            nc.vector.memset(ones1[:], 1.0)
            ps_b = pp.tile([128, O], F32, tag="psb", bufs=1)
            nc.tensor.matmul(ps_b[:], ones1[:], brow[:], start=True, stop=True)
            bias = constp.tile([128, O], F32)
            nc.scalar.copy(bias[:], ps_b[:])

            # ---- on-device im2col: S[j][16g+ch, r*32+w] = xpad[j,ch,r-1+dh,w+dw]
            s_tiles = []
            for j in range(NIN):
                st = sp.tile([97, SF], BF16, name=f"s{j}")
                nc.vector.memset(st[0:96, :], 0.0)
                nc.gpsimd.memset(st[96:97, :], 1.0)
                s_tiles.append(st)
            xv = xall[:].rearrange("p (h w) -> p h w", w=32)
            for j in range(NIN):
                sv = s_tiles[j][:].rearrange("p (r w) -> p r w", w=32)
                for g, (dh, dw) in enumerate(SHIFTS):
                    rlo, rhi = max(0, 1 - dh), min(34, 33 - dh)
                    wlo, whi = max(0, -dw), min(32, 32 - dw)
                    nc.sync.dma_start(
                        sv[16 * g:16 * g + 16, rlo:rhi, wlo:whi],
                        xv[j * 16:(j + 1) * 16,
                           rlo - 1 + dh:rhi - 1 + dh, wlo + dw:whi + dw])

            for c in range(NCHUNK):
                h0 = 4 * c
                # ---------------- conv ----------------
                ps_votes = pp.tile([128, NIN * O], F32, tag="psv", bufs=1)
                ps_pre1 = pp.tile([128, O], F32, tag="psp", bufs=1)
                p0 = (h0 + 1) * 32
                for j in range(NIN):
                    st = s_tiles[j]
                    l96 = st[0:96, p0:p0 + 128]
                    l97 = st[0:97, p0:p0 + 128]
                    l48 = st[0:48, p0 + 64:p0 + 192]
                    vslice = ps_votes[:, j * O:(j + 1) * O]
                    nc.tensor.matmul(vslice, l96, w96[:], start=True, stop=False,
                                     skip_group_check=True)
                    if j == 0:
                        nc.tensor.matmul(ps_pre1[:], l97, w96s[:],
                                         start=True, stop=False,
                                         skip_group_check=True)
                    else:
                        nc.tensor.matmul(ps_pre1[:], l96, w96s[0:96],
                                         start=False, stop=False,
                                         skip_group_check=True)
                    nc.tensor.matmul(vslice, l48, w48[:], start=False, stop=True,
                                     skip_group_check=True)
                    nc.tensor.matmul(ps_pre1[:], l48[0:48], w48s[:],
                                     start=False, stop=(j == NIN - 1),
                                     skip_group_check=True)

                # ---------------- evict ----------------
                votes = wp.tile([128, NIN * O], F32, tag="votes")
                pre1 = smp.tile([128, O], F32, tag="pre1")
                nc.scalar.copy(votes[:], ps_votes[:])
                nc.scalar.copy(pre1[:], ps_pre1[:])

                # ---------------- squash factor f1 from preact1 ----------------
                sqel1 = smp.tile([128, O], F32, tag="sqel1")
                nc.scalar.square(sqel1[:], pre1[:])
                sq1 = smp.tile([128, NC], F32, tag="sq1")
                nc.vector.reduce_sum(
                    sq1[:], sqel1[:].rearrange("p (n d) -> p n d", d=DC),
                    axis=mybir.AxisListType.X)
                f1 = _squash_factor(nc, smp, sq1, "1")

                # ---------------- logits ----------------
                pall = wp.tile([128, NIN * O], F32, tag="pall")
                v3 = votes[:].rearrange("p (j o) -> p j o", j=NIN)
                p1b = pre1[:].unsqueeze(1).broadcast_to([128, NIN, O])
                eng_b = nc.gpsimd if GPSIMD_BMULT else nc.vector
                eng_b.tensor_tensor(
                    pall[:].rearrange("p (j o) -> p j o", j=NIN), v3, p1b, op=OP.mult)
                lg = smp.tile([128, NIN * NC], F32, tag="lg")
                nc.vector.reduce_sum(
                    lg[:], pall[:].rearrange("p (j n d) -> p j n d", n=NC, d=DC),
                    axis=mybir.AxisListType.X)
                logits = smp.tile([128, NIN * NC], F32, tag="logits")
                f1b = f1[:].unsqueeze(1).broadcast_to([128, NIN, NC])
                nc.vector.tensor_tensor(
                    logits[:].rearrange("p (j n) -> p j n", j=NIN),
                    lg[:].rearrange("p (j n) -> p j n", j=NIN), f1b, op=OP.mult)

                # ---------------- softmax over nc ----------------
                ee = smp.tile([128, NIN * NC], F32, tag="ee")
                nc.scalar.activation(ee[:], logits[:], AF.Exp)
                den = smp.tile([128, NIN], F32, tag="den")
                nc.vector.reduce_sum(
                    den[:], ee[:].rearrange("p (j n) -> p j n", j=NIN),
                    axis=mybir.AxisListType.X)
                rcp = smp.tile([128, NIN], F32, tag="rcp")
                nc.vector.reciprocal(rcp[:], den[:])

                # ---------------- preact2 = sum_j route*votes + b ----------------
                route = smp.tile([128, NIN * NC], F32, tag="route")
                rcpb = rcp[:].unsqueeze(2).broadcast_to([128, NIN, NC])
                nc.vector.tensor_tensor(
                    route[:].rearrange("p (j n) -> p j n", j=NIN),
                    ee[:].rearrange("p (j n) -> p j n", j=NIN), rcpb, op=OP.mult)
                p2 = wp.tile([128, NIN * O], F32, tag="p2")
                for j in range(NIN):
                    rj = route[:, j * NC:(j + 1) * NC]
                    rjb = rj.unsqueeze(2).broadcast_to([128, NC, DC])
                    eng = nc.gpsimd if j < GPSIMD_DMULTS else nc.vector
                    eng.tensor_tensor(
                        p2[:, j * O:(j + 1) * O].rearrange("p (n d) -> p n d", n=NC),
                        votes[:, j * O:(j + 1) * O].rearrange("p (n d) -> p n d", n=NC),
                        rjb, op=OP.mult)
                pre2 = smp.tile([128, O], F32, tag="pre2")
                nc.vector.reduce_sum(
                    pre2[:],
                    p2[:].rearrange("p (j n d) -> p n d j", j=NIN, n=NC),
                    axis=mybir.AxisListType.X)
                pre2b = smp.tile([128, O], F32, tag="pre2b")
                nc.vector.tensor_tensor(pre2b[:], pre2[:], bias[:], op=OP.add)

                # ---------------- final squash ----------------
                sqel2 = smp.tile([128, O], F32, tag="sqel2")
                nc.scalar.square(sqel2[:], pre2b[:])
                sq2 = smp.tile([128, NC], F32, tag="sq2")
                nc.vector.reduce_sum(
                    sq2[:], sqel2[:].rearrange("p (n d) -> p n d", d=DC),
                    axis=mybir.AxisListType.X)
                f2 = _squash_factor(nc, smp, sq2, "2")
                act2 = wp.tile([128, O], BF16, tag="act2")
                f2b = f2[:].unsqueeze(2).broadcast_to([128, NC, DC])
                nc.vector.tensor_tensor(
                    act2[:].rearrange("p (n d) -> p n d", n=NC),
                    pre2b[:].rearrange("p (n d) -> p n d", n=NC), f2b, op=OP.mult)

                nc.sync.dma_start(out.ap()[c * 128:(c + 1) * 128], act2[:])

    nc.compile()
    return nc


def _squash_factor(nc, pool, sq, tag):
    """f = sq / ((1+sq) * sqrt(sq+EPS)), shape [128, NC]."""
    sqe = pool.tile([128, NC], F32, name=f"sqe{tag}", tag=f"sqe{tag}")
    nc.vector.tensor_scalar_add(sqe[:], sq[:], EPS)
    rt = pool.tile([128, NC], F32, name=f"rt{tag}", tag=f"rt{tag}")
    nc.scalar.activation(rt[:], sqe[:], AF.Sqrt)
    u = pool.tile([128, NC], F32, name=f"u{tag}", tag=f"u{tag}")
    nc.vector.tensor_scalar_add(u[:], sq[:], 1.0)
    w = pool.tile([128, NC], F32, name=f"w{tag}", tag=f"w{tag}")
    nc.vector.tensor_tensor(w[:], u[:], rt[:], op=OP.mult)
    vr = pool.tile([128, NC], F32, name=f"vr{tag}", tag=f"vr{tag}")
    nc.vector.reciprocal(vr[:], w[:])
    f = pool.tile([128, NC], F32, name=f"f{tag}", tag=f"f{tag}")
    nc.vector.tensor_tensor(f[:], sq[:], vr[:], op=OP.mult)
    return f


def make_global_input(x, W, b):
    """Host prep: [8*201, 1024] bf16 — per-core x slice + packed weights."""
    x = np.asarray(x, dtype=np.float32)
    W = np.asarray(W, dtype=np.float32)
    b = np.asarray(b, dtype=np.float32)

    wpack = np.zeros((WROWS, O), np.float32)
    for g in range(6):
        kh, kw = (0, g) if g < 3 else (1, g - 3)
        wpack[WR96 + 16 * g:WR96 + 16 * g + 16] = W[kh, kw]
    for g in range(3):
        wpack[WR48 + 16 * g:WR48 + 16 * g + 16] = W[2, g]
    bflat = b.reshape(O)
    wpack[WR96S:WR96S + 96] = wpack[WR96:WR96 + 96] / 16.0
    wpack[WR96S + 96] = bflat
    wpack[WR48S:WR48S + 48] = wpack[WR48:WR48 + 48] / 16.0
    wpack[WRB] = bflat

    wslab = np.zeros((WSLAB, 1024), NP_BF16)
    wslab.reshape(-1)[:WROWS * O] = wpack.reshape(-1).astype(NP_BF16)

    # [core, img, ch, h, w] -> per-core [128, 1024]
    xt = np.ascontiguousarray(x.transpose(3, 0, 4, 1, 2)).reshape(
        NCORES, 128, 1024).astype(NP_BF16)
    xg = np.empty((NCORES, XROWS, 1024), NP_BF16)
    xg[:, :128] = xt
    xg[:, 128:] = wslab[None]
    return xg.reshape(NCORES * XROWS, 1024)


def _get_runner():
    if "runner" in _CACHE:
        return _CACHE["runner"]

    nc = _CACHE.get("nc")
    if nc is None:
        nc = _CACHE["nc"] = build_module()

    bass2jax.install_neuronx_cc_hook()
    partition_name = nc.partition_id_tensor.name if nc.partition_id_tensor else None

    in_names, out_names, out_avals = [], [], []
    for alloc in nc.m.functions[0].allocations:
        if not isinstance(alloc, mybir.MemoryLocationSet):
            continue
        name = alloc.memorylocations[0].name
        if alloc.kind == "ExternalInput":
            if name != partition_name:
                in_names.append(name)
        elif alloc.kind == "ExternalOutput":
            out_names.append(name)
            out_avals.append(jax.core.ShapedArray(
                tuple(alloc.tensor_shape), mybir.dt.np(alloc.dtype)))
    assert in_names == ["xin"] and out_names == ["out"], (in_names, out_names)
    all_in = in_names + out_names
    if partition_name:
        all_in.append(partition_name)

    def _body(xarg, oarg):
        operands = [xarg, oarg]
        if partition_name:
            operands.append(bass2jax.partition_id_tensor())
        outs = bass2jax._bass_exec_p.bind(
            *operands, out_avals=tuple(out_avals), in_names=tuple(all_in),
            out_names=tuple(out_names), lowering_input_output_aliases=(),
            sim_require_finite=True, sim_require_nnan=True, nc=nc)
        return outs[0]

    devices = jax.devices()[:NCORES]
    mesh = Mesh(np.asarray(devices), ("core",))
    shard = NamedSharding(mesh, PartitionSpec("core"))
    sharded = jax.jit(
        shard_map(_body, mesh=mesh,
                  in_specs=(PartitionSpec("core"),) * 2,
                  out_specs=PartitionSpec("core"), check_rep=False),
        donate_argnums=(1,), keep_unused=True)
    oshape = tuple(out_avals[0].shape)
    zjit = jax.jit(
        lambda: jnp.zeros((NCORES * oshape[0], *oshape[1:]), out_avals[0].dtype),
        out_shardings=shard)

    def run(xg_np):
        dx = jax.device_put(xg_np, shard)
        return sharded(dx, zjit())

    _CACHE["runner"] = run
    return run


def kernel(x, W, b):
    run = _get_runner()
    xg = make_global_input(x, W, b)
    out = run(xg)
    res = np.asarray(out).astype(np.float32)
    return res.reshape(NCORES, H, W_, NC, DC)
